# revision 1
# baseline (speedup 1.0000x reference)
"""Kabsch loss kernel for Trainium2 (8 NeuronCores, data-parallel over batch).

Math: for each batch b (128 points, 3 dims):
  loss_b = ||xc||_F^2 + ||yc||_F^2 - 2 * nuclear_norm(C),  C = xc^T yc (3x3)
because R = U Vh from SVD(C) gives tr(R^T C) = sum of singular values.
nuclear_norm(C) is computed from the invariants of C (I1=||C||_F^2,
I2 = 2nd invariant of C^T C, e3=|det C|) via Newton iteration on the quartic
  n^4 - 2*I1*n^2 - 8*e3*n + (I1^2 - 4*I2) = 0   (largest root = sigma1+sigma2+sigma3)
normalized so I1 -> 3.

Final output = mean over all (65536, 128, 3) of squared deviation.
"""

import sys

sys.path.insert(0, "/opt/trn_rl_repo")

from contextlib import ExitStack

import numpy as np
import ml_dtypes

import concourse.bass as bass
import concourse.tile as tile
from concourse import bacc, mybir
from concourse.bass_utils import run_bass_kernel_spmd

DT = mybir.dt
ALU = mybir.AluOpType
ACT = mybir.ActivationFunctionType

N_CORES = 8
B_TOTAL = 65536
N_PTS = 128
B_CORE = B_TOTAL // N_CORES  # 8192
F = N_PTS * 3  # 384


def _bv(base_ap, dims):
    """Build an AP reusing base_ap's partition dim + offset with custom free dims."""
    return bass.AP(base_ap.tensor, base_ap.offset, [list(base_ap.ap[0])] + [list(d) for d in dims])


def build_kernel(b_core=B_CORE, n_cores=N_CORES):
    n_tiles = b_core // 128
    assert n_tiles % 8 == 0, "need tiles divisible by 8 (4 per super, 2 halves)"
    n_supers = n_tiles // 4
    half_supers = n_supers // 2
    W = n_tiles // 2  # loss columns per half

    nc = bacc.Bacc("TRN2", target_bir_lowering=False, debug=False, num_devices=n_cores)
    x_d = nc.dram_tensor("x", [b_core, F], DT.float32, kind="ExternalInput").ap()
    y_d = nc.dram_tensor("y", [b_core, F], DT.float32, kind="ExternalInput").ap()
    sel_d = nc.dram_tensor("sel", [128, 128], DT.bfloat16, kind="ExternalInput").ap()
    idb_d = nc.dram_tensor("idb", [128, 128], DT.bfloat16, kind="ExternalInput").ap()
    idf_d = nc.dram_tensor("idf", [128, 128], DT.float32, kind="ExternalInput").ap()
    loss_d = nc.dram_tensor("loss", [128, n_tiles], DT.float32, kind="ExternalOutput").ap()
    ssq_d = nc.dram_tensor("ssq", [128, n_tiles // 2], DT.float32, kind="ExternalOutput").ap()

    with tile.TileContext(nc) as tc:
        with ExitStack() as ctx:
            _kabsch(ctx, tc, x_d, y_d, sel_d, idb_d, idf_d, loss_d, ssq_d,
                    n_tiles, n_supers, half_supers, W)
    nc.compile()
    return nc


def _kabsch(ctx, tc, x_d, y_d, sel_d, idb_d, idf_d, loss_d, ssq_d,
            n_tiles, n_supers, half_supers, W):
    nc = tc.nc
    singles = ctx.enter_context(tc.tile_pool(name="singles", bufs=1))
    loads = ctx.enter_context(tc.tile_pool(name="loads", bufs=3))
    planes = ctx.enter_context(tc.tile_pool(name="planes", bufs=2))
    prods = ctx.enter_context(tc.tile_pool(name="prods", bufs=2))
    statsp = ctx.enter_context(tc.tile_pool(name="statsp", bufs=2))
    junkp = ctx.enter_context(tc.tile_pool(name="junkp", bufs=2))
    fin = ctx.enter_context(tc.tile_pool(name="fin", bufs=1))
    psum = ctx.enter_context(tc.tile_pool(name="psum", bufs=2, space="PSUM"))

    # constants
    sel = singles.tile([128, 128], DT.bfloat16, tag="sel")
    idb = singles.tile([128, 128], DT.bfloat16, tag="idb")
    idf = singles.tile([128, 128], DT.float32, tag="idf")
    nc.sync.dma_start(out=sel, in_=sel_d)
    nc.sync.dma_start(out=idb, in_=idb_d)
    nc.sync.dma_start(out=idf, in_=idf_d)

    # per-half persistent buffers
    ssq_cols = singles.tile([128, 2 * n_supers], DT.float32, tag="ssq_cols", name="ssq_cols")
    stats_h = [singles.tile([128, 15 * W], DT.float32, tag=f"stats{h}", name=f"stats{h}") for h in range(2)]
    loss_h = [singles.tile([128, W], DT.float32, tag=f"loss{h}", name=f"loss{h}") for h in range(2)]

    for s in range(n_supers):
        h = s // half_supers
        sl = s % half_supers  # super index within half

        # ---- load + cast f32 -> bf16 (SWDGE) ----
        xb = loads.tile([128, 4, F], DT.bfloat16, tag="xb")
        yb = loads.tile([128, 4, F], DT.bfloat16, tag="yb")
        nc.gpsimd.dma_start(
            out=xb, in_=x_d[512 * s:512 * (s + 1), :].rearrange("(t p) f -> p t f", p=128))
        nc.gpsimd.dma_start(
            out=yb, in_=y_d[512 * s:512 * (s + 1), :].rearrange("(t p) f -> p t f", p=128))

        # ---- global sum of squares partials (coarse: one column per super) ----
        jx = junkp.tile([128, 4, F], DT.bfloat16, tag="jx")
        jy = junkp.tile([128, 4, F], DT.bfloat16, tag="jy")
        nc.scalar.activation(out=jx, in_=xb, func=ACT.Square,
                             accum_out=ssq_cols[:, s:s + 1])
        nc.vector.scalar_tensor_tensor(
            out=jy, in0=yb, scalar=1.0, in1=yb, op0=ALU.mult, op1=ALU.mult,
            accum_out=ssq_cols[:, n_supers + s:n_supers + s + 1])

        # ---- transposes: [128b, 128i] -> [128i, 128b] planes in PSUM ----
        # one PSUM bank per j: x-plane in cols 0:512, y-plane in cols 512:1024
        pT = [psum.tile([128, 1024], DT.bfloat16, tag=f"pT{j}", name=f"pT{j}") for j in range(3)]
        for t in range(4):
            for j in range(3):
                nc.tensor.transpose(
                    out=pT[j][:, 128 * t:128 * (t + 1)], in_=xb[:, t, j::3], identity=idb)
                nc.tensor.transpose(
                    out=pT[j][:, 512 + 128 * t:512 + 128 * (t + 1)], in_=yb[:, t, j::3],
                    identity=idb)

        # ---- evacuate PSUM -> SBUF (split ACT / DVE) ----
        xT = [planes.tile([128, 512], DT.bfloat16, tag=f"xT{j}", name=f"xT{j}") for j in range(3)]
        yT = [planes.tile([128, 512], DT.bfloat16, tag=f"yT{j}", name=f"yT{j}") for j in range(3)]
        for j in range(3):
            nc.scalar.copy(out=xT[j], in_=pT[j][:, 0:512])
            nc.vector.tensor_copy(out=yT[j], in_=pT[j][:, 512:1024])

        # ---- cross products (DVE, bf16 2x) ----
        pr = {}
        for j in range(3):
            for k in range(3):
                p_ = prods.tile([128, 512], DT.bfloat16, tag=f"pr{j}{k}", name=f"pr{j}{k}")
                nc.vector.tensor_mul(p_, xT[j], yT[k])
                pr[(j, k)] = p_

        # ---- reduction matmuls into stats PSUM (quantity q -> partition q) ----
        # q = 3j+k: G_jk; q = 9+j: sx_j; q = 12+k: sy_k
        pstat = psum.tile([15, 512], DT.float32, tag="stats")
        for q in range(14, -1, -1):
            if q >= 12:
                rhs = yT[q - 12]
            elif q >= 9:
                rhs = xT[q - 9]
            else:
                rhs = pr[(q // 3, q % 3)]
            nc.tensor.matmul(
                out=pstat[0:q + 1, :], lhsT=sel[:, 127 - q:128], rhs=rhs,
                start=(q == 14), stop=(q == 0), skip_group_check=True)

        # ---- evacuate stats, transpose to [batch-partition, quantity] ----
        st_raw = statsp.tile([15, 512], DT.float32, tag="straw")
        nc.scalar.copy(out=st_raw, in_=pstat)
        pchunk = psum.tile([128, 60], DT.float32, tag="stats")
        for tp in range(4):
            nc.tensor.transpose(
                out=pchunk[:, 15 * tp:15 * (tp + 1)],
                in_=st_raw[0:15, 128 * tp:128 * (tp + 1)], identity=idf[0:15, 0:15])
        dstv = stats_h[h][:].rearrange("p (q t) -> p t q", q=15)[:, 4 * sl:4 * (sl + 1), :]
        srcv = pchunk[:].rearrange("p (t q) -> p t q", t=4)
        nc.vector.tensor_copy(out=dstv, in_=srcv)

        if sl == half_supers - 1:
            _final_math(nc, fin, stats_h[h], loss_h[h], W)
            nc.sync.dma_start(out=loss_d[:, h * W:(h + 1) * W], in_=loss_h[h])
    nc.sync.dma_start(out=ssq_d, in_=ssq_cols)


def _final_math(nc, fin, stats, loss, W):
    f32 = DT.float32
    V = nc.vector
    S = nc.scalar

    def T_(tag, mult=1):
        return fin.tile([128, mult * W], f32, tag=tag, name=tag)

    stats_ap = stats[:]
    base = stats_ap.offset
    part = list(stats_ap.ap[0])

    def q_ap(q, n=1):
        """contiguous view of quantities [q, q+n) : [128, n*W]"""
        return stats[:, q * W:(q + n) * W]

    def q_view(q, dims):
        return bass.AP(stats_ap.tensor, base + q * W, [part] + [list(d) for d in dims])

    inv_n = -1.0 / 128.0

    # --- C = G - sx sy^T / N ---
    sp9 = T_("sp9", 9)
    sx_b = q_view(9, [[W, 3], [0, 3], [1, W]])   # (j, k, T)
    sy_b = q_view(12, [[0, 3], [W, 3], [1, W]])
    V.tensor_tensor(out=sp9[:].rearrange("p (j k t) -> p j k t", j=3, k=3),
                    in0=sx_b, in1=sy_b, op=ALU.mult)
    C = T_("C", 9)
    V.scalar_tensor_tensor(out=C, in0=sp9, scalar=inv_n, in1=q_ap(0, 9),
                           op0=ALU.mult, op1=ALU.add)
    Cap = C[:]

    def C_(j, k):
        return C[:, (3 * j + k) * W:(3 * j + k + 1) * W]

    # --- l2 = ssx + ssy - (|sx|^2 + |sy|^2)/N ---
    sq6 = T_("sq6", 6)
    V.tensor_tensor(out=sq6, in0=q_ap(9, 6), in1=q_ap(9, 6), op=ALU.mult)
    sqsum = T_("sqsum")
    V.tensor_reduce(out=sqsum, in_=_bv(sq6[:], [[1, W], [W, 6]]), axis=mybir.AxisListType.X,
                    op=ALU.add)
    l2 = T_("l2")
    V.tensor_scalar_mul(l2, sqsum, inv_n)

    # --- I1 = sum C^2 ---
    csq = T_("csq", 9)
    V.tensor_tensor(out=csq, in0=C, in1=C, op=ALU.mult)
    I1 = T_("I1")
    V.tensor_reduce(out=I1, in_=_bv(csq[:], [[1, W], [W, 9]]), axis=mybir.AxisListType.X,
                    op=ALU.add)

    # --- M = C^T C (9 entries incl. dup), trM2 = sum M^2 ---
    P27 = T_("P27", 27)
    ca = _bv(Cap, [[3 * W, 3], [W, 3], [0, 3], [1, W]])
    cb = _bv(Cap, [[3 * W, 3], [0, 3], [W, 3], [1, W]])
    V.tensor_tensor(out=P27[:].rearrange("p (j a b t) -> p j a b t", j=3, a=3, b=3),
                    in0=ca, in1=cb, op=ALU.mult)
    M9 = T_("M9", 9)
    V.tensor_reduce(out=M9, in_=_bv(P27[:], [[3 * W, 3], [W, 3], [1, W], [9 * W, 3]]),
                    axis=mybir.AxisListType.X, op=ALU.add)
    msq = T_("msq", 9)
    V.tensor_tensor(out=msq, in0=M9, in1=M9, op=ALU.mult)
    trM2 = T_("trM2")
    V.tensor_reduce(out=trM2, in_=_bv(msq[:], [[1, W], [W, 9]]), axis=mybir.AxisListType.X,
                    op=ALU.add)

    # --- I2 = (I1^2 - trM2)/2 ---
    I1sq = T_("I1sq")
    V.tensor_tensor(out=I1sq, in0=I1, in1=I1, op=ALU.mult)
    trM2h = T_("trM2h")
    V.tensor_scalar_mul(trM2h, trM2, 0.5)
    I2 = T_("I2")
    V.scalar_tensor_tensor(out=I2, in0=I1sq, scalar=0.5, in1=trM2h,
                           op0=ALU.mult, op1=ALU.subtract)

    # --- det(C) ---
    ta = T_("ta")
    tb = T_("tb")
    det = T_("det")
    V.tensor_tensor(out=ta, in0=C_(1, 1), in1=C_(2, 2), op=ALU.mult)
    V.tensor_tensor(out=tb, in0=C_(1, 2), in1=C_(2, 1), op=ALU.mult)
    cof = T_("cof")
    V.tensor_tensor(out=cof, in0=ta, in1=tb, op=ALU.subtract)
    V.tensor_tensor(out=det, in0=C_(0, 0), in1=cof, op=ALU.mult)
    V.tensor_tensor(out=ta, in0=C_(1, 0), in1=C_(2, 2), op=ALU.mult)
    V.tensor_tensor(out=tb, in0=C_(1, 2), in1=C_(2, 0), op=ALU.mult)
    V.tensor_tensor(out=cof, in0=ta, in1=tb, op=ALU.subtract)
    V.tensor_tensor(out=cof, in0=C_(0, 1), in1=cof, op=ALU.mult)
    V.tensor_tensor(out=det, in0=det, in1=cof, op=ALU.subtract)
    V.tensor_tensor(out=ta, in0=C_(1, 0), in1=C_(2, 1), op=ALU.mult)
    V.tensor_tensor(out=tb, in0=C_(1, 1), in1=C_(2, 0), op=ALU.mult)
    V.tensor_tensor(out=cof, in0=ta, in1=tb, op=ALU.subtract)
    V.tensor_tensor(out=cof, in0=C_(0, 2), in1=cof, op=ALU.mult)
    V.tensor_tensor(out=det, in0=det, in1=cof, op=ALU.add)
    e3 = T_("e3")
    S.activation(out=e3, in_=det, func=ACT.Abs)

    # --- normalize: u = 3/I1 ---
    I1c = T_("I1c")
    V.tensor_scalar_max(I1c, I1, 1e-20)
    u = T_("u")
    V.reciprocal(out=u, in_=I1c)
    V.tensor_scalar_mul(u, u, 3.0)
    usq = T_("usq")
    V.tensor_tensor(out=usq, in0=u, in1=u, op=ALU.mult)
    I2n = T_("I2n")
    V.tensor_tensor(out=I2n, in0=I2, in1=usq, op=ALU.mult)
    V.tensor_scalar_max(I2n, I2n, 0.0)
    su = T_("su")
    S.activation(out=su, in_=u, func=ACT.Sqrt)
    e3n = T_("e3n")
    V.tensor_tensor(out=e3n, in0=e3, in1=u, op=ALU.mult)
    V.tensor_tensor(out=e3n, in0=e3n, in1=su, op=ALU.mult)
    E8 = T_("E8")
    V.tensor_scalar_mul(E8, e3n, 8.0)
    c0 = T_("c0")
    V.tensor_scalar(out=c0, in0=I2n, scalar1=-4.0, scalar2=9.0, op0=ALU.mult, op1=ALU.add)

    # --- Newton init: n = sqrt(3 + 2*sqrt(I2n)) ---
    b3 = fin.tile([128, 1], f32, tag="b3", name="b3")
    V.memset(b3, 3.0)
    sqi = T_("sqi")
    S.activation(out=sqi, in_=I2n, func=ACT.Sqrt)
    n = T_("n")
    S.activation(out=n, in_=sqi, func=ACT.Sqrt, bias=b3[:, 0:1], scale=2.0)

    # --- Newton iterations on n^4 - 6n^2 - 8 e3n n + c0 ---
    t1 = T_("t1")
    t3 = T_("t3")
    s1 = T_("s1")
    f0 = T_("f0")
    fv = T_("fv")
    av = T_("av")
    fp = T_("fp")
    rp = T_("rp")
    dd = T_("dd")
    for it in range(4):
        V.tensor_tensor(out=t1, in0=n, in1=n, op=ALU.mult)
        V.scalar_tensor_tensor(out=t3, in0=t1, scalar=-6.0, in1=n,
                               op0=ALU.add, op1=ALU.mult)  # (n^2-6)*n
        V.scalar_tensor_tensor(out=s1, in0=E8, scalar=-1.0, in1=t3,
                               op0=ALU.mult, op1=ALU.add)  # t3 - E8
        V.tensor_tensor(out=f0, in0=s1, in1=n, op=ALU.mult)
        V.tensor_tensor(out=fv, in0=f0, in1=c0, op=ALU.add)
        V.scalar_tensor_tensor(out=av, in0=n, scalar=3.0, in1=t3,
                               op0=ALU.mult, op1=ALU.add)  # n^3 - 3n
        V.scalar_tensor_tensor(out=fp, in0=av, scalar=4.0, in1=E8,
                               op0=ALU.mult, op1=ALU.subtract)  # 4n^3-12n-8e
        V.tensor_scalar_max(fp, fp, 1e-5)
        V.reciprocal(out=rp, in_=fp)
        V.tensor_tensor(out=dd, in0=fv, in1=rp, op=ALU.mult)
        V.tensor_tensor(out=n, in0=n, in1=dd, op=ALU.subtract)
        if it == 0:
            V.tensor_scalar_min(n, n, 3.01)
            V.tensor_scalar_max(n, n, 1.70)

    # --- un-normalize: s = sqrt(I1/3), one Newton refinement for sqrt accuracy ---
    vv = T_("vv")
    V.tensor_scalar_mul(vv, I1, 1.0 / 3.0)
    V.tensor_scalar_max(vv, vv, 1e-30)
    s0 = T_("s0")
    S.activation(out=s0, in_=vv, func=ACT.Sqrt)
    rs = T_("rs")
    V.reciprocal(out=rs, in_=s0)
    V.tensor_tensor(out=rs, in0=vv, in1=rs, op=ALU.mult)   # vv/s0
    V.tensor_tensor(out=rs, in0=rs, in1=s0, op=ALU.add)
    V.tensor_scalar_mul(rs, rs, 0.5)                       # refined sqrt

    # --- loss = l2 - 2 * n * s ---
    V.tensor_tensor(out=n, in0=n, in1=rs, op=ALU.mult)
    V.scalar_tensor_tensor(out=loss[:], in0=n, scalar=-2.0, in1=l2,
                           op0=ALU.mult, op1=ALU.add)


# ---------------------------------------------------------------------------
# host glue
# ---------------------------------------------------------------------------


class Runner:
    """Cached jitted shard_map executor for repeated invocations (timing)."""

    def __init__(self, nc, n_cores=N_CORES):
        import jax
        from jax.experimental.shard_map import shard_map
        from jax.sharding import Mesh, PartitionSpec
        from concourse import bass2jax
        from concourse import mybir as _mybir

        bass2jax.install_neuronx_cc_hook()
        self.nc = nc
        self.n_cores = n_cores
        partition_name = nc.partition_id_tensor.name if nc.partition_id_tensor else None
        in_names, out_names, out_avals, zero_outs = [], [], [], []
        for alloc in nc.m.functions[0].allocations:
            if not isinstance(alloc, _mybir.MemoryLocationSet):
                continue
            name = alloc.memorylocations[0].name
            if alloc.kind == "ExternalInput":
                if name != partition_name:
                    in_names.append(name)
            elif alloc.kind == "ExternalOutput":
                out_names.append(name)
                shape = tuple(alloc.tensor_shape)
                dtype = _mybir.dt.np(alloc.dtype)
                out_avals.append(jax.core.ShapedArray(shape, dtype))
                zero_outs.append(np.zeros(shape, dtype))
        self.in_names = list(in_names)
        self.out_names = out_names
        self.zero_outs = zero_outs
        n_params = len(in_names)
        n_outs = len(out_avals)
        all_in_names = in_names + out_names
        if partition_name is not None:
            all_in_names = all_in_names + [partition_name]

        def _body(*args):
            operands = list(args)
            if partition_name is not None:
                operands.append(bass2jax.partition_id_tensor())
            outs = bass2jax._bass_exec_p.bind(
                *operands,
                out_avals=tuple(out_avals),
                in_names=tuple(all_in_names),
                out_names=tuple(out_names),
                lowering_input_output_aliases=(),
                sim_require_finite=True,
                sim_require_nnan=True,
                nc=nc,
            )
            return tuple(outs)

        devices = jax.devices()[:n_cores]
        mesh = Mesh(np.asarray(devices), ("core",))
        self.mesh = mesh
        in_specs = (PartitionSpec("core"),) * (n_params + n_outs)
        out_specs = (PartitionSpec("core"),) * n_outs
        self.fn = jax.jit(
            shard_map(_body, mesh=mesh, in_specs=in_specs, out_specs=out_specs,
                      check_rep=False),
            keep_unused=True,
        )

    def prep(self, in_maps, device_put=True):
        """in_maps: list of per-core dicts -> concatenated arg list (device-resident)."""
        concat = [
            np.concatenate([np.asarray(in_maps[c][n]) for c in range(self.n_cores)], axis=0)
            for n in self.in_names
        ]
        concat += [
            np.zeros((self.n_cores * z.shape[0], *z.shape[1:]), z.dtype)
            for z in self.zero_outs
        ]
        if device_put:
            import jax
            from jax.sharding import NamedSharding, PartitionSpec

            sh = NamedSharding(self.mesh, PartitionSpec("core"))
            concat = [jax.device_put(a, sh) for a in concat]
            jax.block_until_ready(concat)
        return concat

    def __call__(self, args):
        return self.fn(*args)


_NC_CACHE = {}


def _get_nc(b_core=B_CORE):
    if b_core not in _NC_CACHE:
        _NC_CACHE[b_core] = build_kernel(b_core)
    return _NC_CACHE[b_core]


def _consts():
    sel = np.zeros((128, 128), ml_dtypes.bfloat16)
    sel[:, 127] = 1.0
    idb = np.eye(128, dtype=ml_dtypes.bfloat16)
    idf = np.eye(128, dtype=np.float32)
    return sel, idb, idf


def run_cores(x, y, b_core=B_CORE, n_cores=N_CORES, nc=None):
    """x, y: (n_cores*b_core, 128, 3) float32 -> list of per-core loss grids."""
    if nc is None:
        nc = _get_nc(b_core)
    sel, idb, idf = _consts()
    xs = np.ascontiguousarray(x, dtype=np.float32).reshape(n_cores, b_core, F)
    ys = np.ascontiguousarray(y, dtype=np.float32).reshape(n_cores, b_core, F)
    in_maps = [
        {"x": xs[c], "y": ys[c], "sel": sel, "idb": idb, "idf": idf}
        for c in range(n_cores)
    ]
    res = run_bass_kernel_spmd(nc, in_maps, core_ids=list(range(n_cores)))
    return [(res.results[c]["loss"], res.results[c]["ssq"]) for c in range(n_cores)]


def kernel(x, y):
    """Full-input entry point: x, y (65536, 128, 3) float32 -> scalar float32."""
    grids = run_cores(np.asarray(x), np.asarray(y))
    total = sum(
        g.astype(np.float64).sum() + q.astype(np.float64).sum() for g, q in grids
    )
    return np.float32(total / (B_TOTAL * N_PTS * 3))



# revision 14
# speedup vs baseline: 4.2610x; 4.2610x over previous
"""Kabsch loss kernel for Trainium2 (8 NeuronCores, data-parallel over batch).

Math: for each batch b (128 points, 3 dims):
  loss_b = ||xc||_F^2 + ||yc||_F^2 - 2 * nuclear_norm(C),  C = xc^T yc (3x3)
because R = U Vh from SVD(C) gives tr(R^T C) = sum of singular values.
nuclear_norm(C) is computed from the invariants of C (I1=||C||_F^2,
I2 = 2nd invariant of C^T C, e3=|det C|) via Newton iteration on the quartic
  n^4 - 2*I1*n^2 - 8*e3*n + (I1^2 - 4*I2) = 0   (largest root = sigma1+sigma2+sigma3)
normalized so I1 -> 3.

Final output = mean over all (65536, 128, 3) of squared deviation.
"""

import sys

sys.path.insert(0, "/opt/trn_rl_repo")

from contextlib import ExitStack

import numpy as np
import ml_dtypes

import concourse.bass as bass
import concourse.tile as tile
from concourse import bacc, mybir
from concourse.bass_utils import run_bass_kernel_spmd

DT = mybir.dt
ALU = mybir.AluOpType
ACT = mybir.ActivationFunctionType

N_CORES = 8
B_TOTAL = 65536
N_PTS = 128
B_CORE = B_TOTAL // N_CORES  # 8192
F = N_PTS * 3  # 384


def _bv(base_ap, dims):
    """Build an AP reusing base_ap's partition dim + offset with custom free dims."""
    return bass.AP(base_ap.tensor, base_ap.offset, [list(base_ap.ap[0])] + [list(d) for d in dims])


def build_kernel(b_core=B_CORE, n_cores=N_CORES):
    n_tiles = b_core // 128
    assert n_tiles % 8 == 0, "need tiles divisible by 8 (4 per super, 2 halves)"
    n_supers = n_tiles // 4
    half_supers = n_supers // 2
    W = n_tiles // 2  # loss columns per half

    nc = bacc.Bacc("TRN2", target_bir_lowering=False, debug=False, num_devices=n_cores)
    x_d = nc.dram_tensor("x", [b_core, F], DT.float32, kind="ExternalInput").ap()
    y_d = nc.dram_tensor("y", [b_core, F], DT.float32, kind="ExternalInput").ap()
    sel_d = nc.dram_tensor("sel", [128, 128], DT.bfloat16, kind="ExternalInput").ap()
    idb_d = nc.dram_tensor("idb", [128, 128], DT.bfloat16, kind="ExternalInput").ap()
    idf_d = nc.dram_tensor("idf", [128, 128], DT.float32, kind="ExternalInput").ap()
    loss_d = nc.dram_tensor("loss", [128, n_tiles], DT.float32, kind="ExternalOutput").ap()
    ssq_d = nc.dram_tensor("ssq", [128, n_tiles // 2], DT.float32, kind="ExternalOutput").ap()

    with tile.TileContext(nc) as tc:
        with ExitStack() as ctx:
            _kabsch(ctx, tc, x_d, y_d, sel_d, idb_d, idf_d, loss_d, ssq_d,
                    n_tiles, n_supers, half_supers, W)
    nc.compile()
    return nc


def _kabsch(ctx, tc, x_d, y_d, sel_d, idb_d, idf_d, loss_d, ssq_d,
            n_tiles, n_supers, half_supers, W):
    nc = tc.nc
    singles = ctx.enter_context(tc.tile_pool(name="singles", bufs=1))
    loads = ctx.enter_context(tc.tile_pool(name="loads", bufs=3))
    planes = ctx.enter_context(tc.tile_pool(name="planes", bufs=2))
    prods = ctx.enter_context(tc.tile_pool(name="prods", bufs=2))
    statsp = ctx.enter_context(tc.tile_pool(name="statsp", bufs=2))
    junkp = ctx.enter_context(tc.tile_pool(name="junkp", bufs=2))
    fin = ctx.enter_context(tc.tile_pool(name="fin", bufs=1))
    psum = ctx.enter_context(tc.tile_pool(name="psum", bufs=2, space="PSUM"))

    # constants
    sel = singles.tile([128, 128], DT.bfloat16, tag="sel")
    idb = singles.tile([128, 128], DT.bfloat16, tag="idb")
    idf = singles.tile([128, 128], DT.float32, tag="idf")
    nc.sync.dma_start(out=sel, in_=sel_d)
    nc.sync.dma_start(out=idb, in_=idb_d)
    nc.sync.dma_start(out=idf, in_=idf_d)

    # per-half persistent buffers
    ssq_cols = singles.tile([128, 2 * n_supers], DT.float32, tag="ssq_cols", name="ssq_cols")
    stats_h = [singles.tile([128, 15 * W], DT.float32, tag=f"stats{h}", name=f"stats{h}") for h in range(2)]
    loss_h = [singles.tile([128, W], DT.float32, tag=f"loss{h}", name=f"loss{h}") for h in range(2)]

    for s in range(n_supers):
        h = s // half_supers
        sl = s % half_supers  # super index within half

        # ---- load + cast f32 -> bf16 (SWDGE) ----
        xb = loads.tile([128, 4, F], DT.bfloat16, tag="xb")
        yb = loads.tile([128, 4, F], DT.bfloat16, tag="yb")
        nc.gpsimd.dma_start(
            out=xb, in_=x_d[512 * s:512 * (s + 1), :].rearrange("(t p) f -> p t f", p=128))
        nc.gpsimd.dma_start(
            out=yb, in_=y_d[512 * s:512 * (s + 1), :].rearrange("(t p) f -> p t f", p=128))

        # ---- global sum of squares partials (x on ACT, y on DVE) ----
        jx = junkp.tile([128, 4, F], DT.bfloat16, tag="jx")
        jy = junkp.tile([128, 4, F], DT.bfloat16, tag="jy")
        nc.scalar.activation(out=jx, in_=xb, func=ACT.Square,
                             accum_out=ssq_cols[:, s:s + 1])
        nc.vector.scalar_tensor_tensor(
            out=jy, in0=yb, scalar=1.0, in1=yb, op0=ALU.mult, op1=ALU.mult,
            accum_out=ssq_cols[:, n_supers + s:n_supers + s + 1])

        # ---- transposes: [128b, 128i] -> [128i, 128b] planes in PSUM ----
        # one PSUM bank per j: x-plane in cols 0:512, y-plane in cols 512:1024
        pT = [psum.tile([128, 1024], DT.bfloat16, tag=f"pT{j}", name=f"pT{j}") for j in range(3)]
        for t in range(4):
            for j in range(3):
                nc.tensor.transpose(
                    out=pT[j][:, 128 * t:128 * (t + 1)], in_=xb[:, t, j::3], identity=idb)
                nc.tensor.transpose(
                    out=pT[j][:, 512 + 128 * t:512 + 128 * (t + 1)], in_=yb[:, t, j::3],
                    identity=idb)

        # ---- evacuate PSUM -> SBUF (split ACT / DVE, as measured-fastest) ----
        xT = [planes.tile([128, 512], DT.bfloat16, tag=f"xT{j}", name=f"xT{j}") for j in range(3)]
        yT = [planes.tile([128, 512], DT.bfloat16, tag=f"yT{j}", name=f"yT{j}") for j in range(3)]
        for j in range(3):
            nc.scalar.copy(out=xT[j], in_=pT[j][:, 0:512])
            nc.vector.tensor_copy(out=yT[j], in_=pT[j][:, 512:1024])

        # ---- cross products (DVE, bf16 2x) ----
        pr = {}
        for j in range(3):
            for k in range(3):
                p_ = prods.tile([128, 512], DT.bfloat16, tag=f"pr{j}{k}", name=f"pr{j}{k}")
                nc.vector.tensor_mul(p_, xT[j], yT[k])
                pr[(j, k)] = p_

        # ---- reduction matmuls into stats PSUM (quantity q -> partition q) ----
        # q = 3j+k: G_jk; q = 9+j: sx_j; q = 12+k: sy_k
        pstat = psum.tile([15, 512], DT.float32, tag="stats")
        for q in range(14, -1, -1):
            if q >= 12:
                rhs = yT[q - 12]
            elif q >= 9:
                rhs = xT[q - 9]
            else:
                rhs = pr[(q // 3, q % 3)]
            nc.tensor.matmul(
                out=pstat[0:q + 1, :], lhsT=sel[:, 127 - q:128], rhs=rhs,
                start=(q == 14), stop=(q == 0), skip_group_check=True)

        # ---- evacuate stats, transpose to [batch-partition, quantity] ----
        st_raw = statsp.tile([15, 512], DT.float32, tag="straw")
        nc.scalar.copy(out=st_raw, in_=pstat)
        pchunk = psum.tile([128, 60], DT.float32, tag="stats")
        for tp in range(4):
            nc.tensor.transpose(
                out=pchunk[:, 15 * tp:15 * (tp + 1)],
                in_=st_raw[0:15, 128 * tp:128 * (tp + 1)], identity=idf[0:15, 0:15])
        dstv = stats_h[h][:].rearrange("p (q t) -> p t q", q=15)[:, 4 * sl:4 * (sl + 1), :]
        srcv = pchunk[:].rearrange("p (t q) -> p t q", t=4)
        nc.vector.tensor_copy(out=dstv, in_=srcv)

        if sl == half_supers - 1:
            _final_math(nc, fin, stats_h[h], loss_h[h], W)
            nc.sync.dma_start(out=loss_d[:, h * W:(h + 1) * W], in_=loss_h[h])
    nc.sync.dma_start(out=ssq_d, in_=ssq_cols)


def _final_math(nc, fin, stats, loss, W):
    """Per-batch: C = G - sx sy^T/N; invariants of C via cofactors; nuclear
    norm n = sigma1+sigma2+sigma3 from the coupled fixed point
        u = sqrt(I2 + 2|det| n),   n = sqrt(I1 + 2u)
    (contraction factor <= 1/9 by AM-GM), then loss col = -|s|^2/N - 2n.
    Division-free; ACT uses only Sqrt/Abs (one activation table)."""
    f32 = DT.float32
    V = nc.vector
    S = nc.scalar

    def T_(tag, mult=1):
        return fin.tile([128, mult * W], f32, tag=tag, name=tag)

    stats_ap = stats[:]
    base = stats_ap.offset
    part = list(stats_ap.ap[0])

    def q_ap(q, n=1):
        """contiguous view of quantities [q, q+n) : [128, n*W]"""
        return stats[:, q * W:(q + n) * W]

    def q_view(q, dims):
        return bass.AP(stats_ap.tensor, base + q * W, [part] + [list(d) for d in dims])

    inv_n = -1.0 / 128.0

    # --- C = G - sx sy^T / N ---
    sp9 = T_("sp9", 9)
    sx_b = q_view(9, [[W, 3], [0, 3], [1, W]])   # (j, k, T)
    sy_b = q_view(12, [[0, 3], [W, 3], [1, W]])
    V.tensor_tensor(out=sp9[:].rearrange("p (j k t) -> p j k t", j=3, k=3),
                    in0=sx_b, in1=sy_b, op=ALU.mult)
    C = T_("C", 9)
    V.scalar_tensor_tensor(out=C, in0=sp9, scalar=inv_n, in1=q_ap(0, 9),
                           op0=ALU.mult, op1=ALU.add)

    # --- l2 = -(|sx|^2 + |sy|^2)/N  (raw ssq added globally on host) ---
    sq6 = T_("sq6", 6)
    V.tensor_tensor(out=sq6, in0=q_ap(9, 6), in1=q_ap(9, 6), op=ALU.mult)
    sqsum = T_("sqsum")
    V.tensor_reduce(out=sqsum, in_=_bv(sq6[:], [[1, W], [W, 6]]), axis=mybir.AxisListType.X,
                    op=ALU.add)
    l2 = T_("l2")
    V.tensor_scalar_mul(l2, sqsum, inv_n)

    # --- I1 = sum C^2 = tr(C^T C) ---
    csq = T_("csq", 9)
    V.tensor_tensor(out=csq, in0=C, in1=C, op=ALU.mult)
    I1 = T_("I1")
    V.tensor_reduce(out=I1, in_=_bv(csq[:], [[1, W], [W, 9]]), axis=mybir.AxisListType.X,
                    op=ALU.add)

    # --- E[a,b] = C[a%3, b%3] for a,b in 0..4 (mod-3 wraparound buffer) ---
    E = T_("E", 25)
    E_ap = E[:]
    E_base = E_ap.offset
    E_part = list(E_ap.ap[0])

    def E_view(a0, b0, na, nb):
        return bass.AP(E_ap.tensor, E_base + (5 * a0 + b0) * W,
                       [E_part, [5 * W, na], [W, nb], [1, W]])

    V.tensor_copy(out=E_view(0, 0, 3, 3), in_=_bv(C[:], [[3 * W, 3], [W, 3], [1, W]]))
    V.tensor_copy(out=E_view(3, 0, 2, 3), in_=E_view(0, 0, 2, 3))
    V.tensor_copy(out=E_view(0, 3, 5, 2), in_=E_view(0, 0, 5, 2))

    # --- signed cofactors: cof_jk = E[j+1,k+1]E[j+2,k+2] - E[j+1,k+2]E[j+2,k+1] ---
    ta = T_("ta", 9)
    tb = T_("tb", 9)
    cof = T_("cof", 9)

    def dense9(t):
        return t[:].rearrange("p (j k t) -> p j k t", j=3, k=3)

    V.tensor_tensor(out=dense9(ta), in0=E_view(1, 1, 3, 3), in1=E_view(2, 2, 3, 3),
                    op=ALU.mult)
    V.tensor_tensor(out=dense9(tb), in0=E_view(1, 2, 3, 3), in1=E_view(2, 1, 3, 3),
                    op=ALU.mult)
    V.tensor_tensor(out=cof, in0=ta, in1=tb, op=ALU.subtract)

    # --- det = sum_k C[0,k] cof[0,k];  I2 = sum cof^2 = |adj C|_F^2 ---
    dm = T_("dm", 3)
    V.tensor_tensor(out=dm, in0=C[:, 0:3 * W], in1=cof[:, 0:3 * W], op=ALU.mult)
    det = T_("det")
    V.tensor_reduce(out=det, in_=_bv(dm[:], [[1, W], [W, 3]]), axis=mybir.AxisListType.X,
                    op=ALU.add)
    d2 = T_("d2")
    S.activation(out=d2, in_=det, func=ACT.Abs, scale=2.0)  # 2|det|
    cq = T_("cq", 9)
    V.tensor_tensor(out=cq, in0=cof, in1=cof, op=ALU.mult)
    I2 = T_("I2")
    V.tensor_reduce(out=I2, in_=_bv(cq[:], [[1, W], [W, 9]]), axis=mybir.AxisListType.X,
                    op=ALU.add)

    # --- fixed point for nuclear norm ---
    u = T_("u")
    n = T_("n")
    t1 = T_("t1")
    S.activation(out=u, in_=I2, func=ACT.Sqrt)
    V.scalar_tensor_tensor(out=t1, in0=u, scalar=2.0, in1=I1, op0=ALU.mult, op1=ALU.add)
    S.activation(out=n, in_=t1, func=ACT.Sqrt)
    for _ in range(4):
        V.tensor_tensor(out=t1, in0=d2, in1=n, op=ALU.mult)
        V.tensor_tensor(out=t1, in0=t1, in1=I2, op=ALU.add)
        S.activation(out=u, in_=t1, func=ACT.Sqrt)
        V.scalar_tensor_tensor(out=t1, in0=u, scalar=2.0, in1=I1,
                               op0=ALU.mult, op1=ALU.add)
        S.activation(out=n, in_=t1, func=ACT.Sqrt)

    # --- loss = l2 - 2 n ---
    V.scalar_tensor_tensor(out=loss[:], in0=n, scalar=-2.0, in1=l2,
                           op0=ALU.mult, op1=ALU.add)


# ---------------------------------------------------------------------------
# host glue
# ---------------------------------------------------------------------------


class Runner:
    """Cached jitted shard_map executor for repeated invocations (timing)."""

    def __init__(self, nc, n_cores=N_CORES):
        import jax
        from jax.experimental.shard_map import shard_map
        from jax.sharding import Mesh, PartitionSpec
        from concourse import bass2jax
        from concourse import mybir as _mybir

        bass2jax.install_neuronx_cc_hook()
        self.nc = nc
        self.n_cores = n_cores
        partition_name = nc.partition_id_tensor.name if nc.partition_id_tensor else None
        in_names, out_names, out_avals, zero_outs = [], [], [], []
        for alloc in nc.m.functions[0].allocations:
            if not isinstance(alloc, _mybir.MemoryLocationSet):
                continue
            name = alloc.memorylocations[0].name
            if alloc.kind == "ExternalInput":
                if name != partition_name:
                    in_names.append(name)
            elif alloc.kind == "ExternalOutput":
                out_names.append(name)
                shape = tuple(alloc.tensor_shape)
                dtype = _mybir.dt.np(alloc.dtype)
                out_avals.append(jax.core.ShapedArray(shape, dtype))
                zero_outs.append(np.zeros(shape, dtype))
        self.in_names = list(in_names)
        self.out_names = out_names
        self.zero_outs = zero_outs
        n_params = len(in_names)
        n_outs = len(out_avals)
        all_in_names = in_names + out_names
        if partition_name is not None:
            all_in_names = all_in_names + [partition_name]

        def _body(*args):
            operands = list(args)
            if partition_name is not None:
                operands.append(bass2jax.partition_id_tensor())
            outs = bass2jax._bass_exec_p.bind(
                *operands,
                out_avals=tuple(out_avals),
                in_names=tuple(all_in_names),
                out_names=tuple(out_names),
                lowering_input_output_aliases=(),
                sim_require_finite=True,
                sim_require_nnan=True,
                nc=nc,
            )
            return tuple(outs)

        devices = jax.devices()[:n_cores]
        mesh = Mesh(np.asarray(devices), ("core",))
        self.mesh = mesh
        in_specs = (PartitionSpec("core"),) * (n_params + n_outs)
        out_specs = (PartitionSpec("core"),) * n_outs
        self.fn = jax.jit(
            shard_map(_body, mesh=mesh, in_specs=in_specs, out_specs=out_specs,
                      check_rep=False),
            keep_unused=True,
        )

    def prep(self, in_maps, device_put=True):
        """in_maps: list of per-core dicts -> concatenated arg list (device-resident)."""
        concat = [
            np.concatenate([np.asarray(in_maps[c][n]) for c in range(self.n_cores)], axis=0)
            for n in self.in_names
        ]
        concat += [
            np.zeros((self.n_cores * z.shape[0], *z.shape[1:]), z.dtype)
            for z in self.zero_outs
        ]
        if device_put:
            import jax
            from jax.sharding import NamedSharding, PartitionSpec

            sh = NamedSharding(self.mesh, PartitionSpec("core"))
            concat = [jax.device_put(a, sh) for a in concat]
            jax.block_until_ready(concat)
        return concat

    def __call__(self, args):
        return self.fn(*args)


_NC_CACHE = {}


def _get_nc(b_core=B_CORE):
    if b_core not in _NC_CACHE:
        _NC_CACHE[b_core] = build_kernel(b_core)
    return _NC_CACHE[b_core]


def _consts():
    sel = np.zeros((128, 128), ml_dtypes.bfloat16)
    sel[:, 127] = 1.0
    idb = np.eye(128, dtype=ml_dtypes.bfloat16)
    idf = np.eye(128, dtype=np.float32)
    return sel, idb, idf


def run_cores(x, y, b_core=B_CORE, n_cores=N_CORES, nc=None):
    """x, y: (n_cores*b_core, 128, 3) float32 -> list of per-core loss grids."""
    if nc is None:
        nc = _get_nc(b_core)
    sel, idb, idf = _consts()
    xs = np.ascontiguousarray(x, dtype=np.float32).reshape(n_cores, b_core, F)
    ys = np.ascontiguousarray(y, dtype=np.float32).reshape(n_cores, b_core, F)
    in_maps = [
        {"x": xs[c], "y": ys[c], "sel": sel, "idb": idb, "idf": idf}
        for c in range(n_cores)
    ]
    res = run_bass_kernel_spmd(nc, in_maps, core_ids=list(range(n_cores)))
    return [(res.results[c]["loss"], res.results[c]["ssq"]) for c in range(n_cores)]


def kernel(x, y):
    """Full-input entry point: x, y (65536, 128, 3) float32 -> scalar float32."""
    grids = run_cores(np.asarray(x), np.asarray(y))
    total = sum(
        g.astype(np.float64).sum() + q.astype(np.float64).sum() for g, q in grids
    )
    return np.float32(total / (B_TOTAL * N_PTS * 3))



# revision 15
# speedup vs baseline: 7.0459x; 1.6536x over previous
"""Kabsch loss kernel for Trainium2 (8 NeuronCores, data-parallel over batch).

Math: for each batch b (128 points, 3 dims):
  loss_b = ||xc||_F^2 + ||yc||_F^2 - 2 * nuclear_norm(C),  C = xc^T yc (3x3)
because R = U Vh from SVD(C) gives tr(R^T C) = sum of singular values.
nuclear_norm(C) is computed from the invariants of C (I1=||C||_F^2,
I2 = 2nd invariant of C^T C, e3=|det C|) via Newton iteration on the quartic
  n^4 - 2*I1*n^2 - 8*e3*n + (I1^2 - 4*I2) = 0   (largest root = sigma1+sigma2+sigma3)
normalized so I1 -> 3.

Final output = mean over all (65536, 128, 3) of squared deviation.
"""

import sys

sys.path.insert(0, "/opt/trn_rl_repo")

from contextlib import ExitStack

import numpy as np
import ml_dtypes

import concourse.bass as bass
import concourse.tile as tile
from concourse import bacc, mybir
from concourse.bass_utils import run_bass_kernel_spmd

DT = mybir.dt
ALU = mybir.AluOpType
ACT = mybir.ActivationFunctionType

N_CORES = 8
B_TOTAL = 65536
N_PTS = 128
B_CORE = B_TOTAL // N_CORES  # 8192
F = N_PTS * 3  # 384


def _bv(base_ap, dims):
    """Build an AP reusing base_ap's partition dim + offset with custom free dims."""
    return bass.AP(base_ap.tensor, base_ap.offset, [list(base_ap.ap[0])] + [list(d) for d in dims])


def build_kernel(b_core=B_CORE, n_cores=N_CORES):
    n_tiles = b_core // 128
    assert n_tiles % 8 == 0, "need tiles divisible by 8 (4 per super, 2 halves)"
    n_supers = n_tiles // 4
    half_supers = n_supers // 2
    W = n_tiles // 2  # loss columns per half

    nc = bacc.Bacc("TRN2", target_bir_lowering=False, debug=False, num_devices=n_cores)
    x_d = nc.dram_tensor("x", [b_core, F], DT.float32, kind="ExternalInput").ap()
    y_d = nc.dram_tensor("y", [b_core, F], DT.float32, kind="ExternalInput").ap()
    sel_d = nc.dram_tensor("sel", [128, 128], DT.bfloat16, kind="ExternalInput").ap()
    idb_d = nc.dram_tensor("idb", [128, 128], DT.bfloat16, kind="ExternalInput").ap()
    idf_d = nc.dram_tensor("idf", [128, 128], DT.float32, kind="ExternalInput").ap()
    loss_d = nc.dram_tensor("loss", [128, n_tiles], DT.float32, kind="ExternalOutput").ap()
    ssq_d = nc.dram_tensor("ssq", [128, n_tiles // 2], DT.float32, kind="ExternalOutput").ap()

    with tile.TileContext(nc) as tc:
        with ExitStack() as ctx:
            _kabsch(ctx, tc, x_d, y_d, sel_d, idb_d, idf_d, loss_d, ssq_d,
                    n_tiles, n_supers, half_supers, W)
    nc.compile()
    return nc


def _kabsch(ctx, tc, x_d, y_d, sel_d, idb_d, idf_d, loss_d, ssq_d,
            n_tiles, n_supers, half_supers, W):
    nc = tc.nc
    singles = ctx.enter_context(tc.tile_pool(name="singles", bufs=1))
    loads = ctx.enter_context(tc.tile_pool(name="loads", bufs=3))
    planes = ctx.enter_context(tc.tile_pool(name="planes", bufs=2))
    prods = ctx.enter_context(tc.tile_pool(name="prods", bufs=2))
    statsp = ctx.enter_context(tc.tile_pool(name="statsp", bufs=2))
    junkp = ctx.enter_context(tc.tile_pool(name="junkp", bufs=2))
    fin = ctx.enter_context(tc.tile_pool(name="fin", bufs=1))
    psum = ctx.enter_context(tc.tile_pool(name="psum", bufs=2, space="PSUM"))

    # constants
    sel = singles.tile([128, 128], DT.bfloat16, tag="sel")
    idb = singles.tile([128, 128], DT.bfloat16, tag="idb")
    idf = singles.tile([128, 128], DT.float32, tag="idf")
    nc.sync.dma_start(out=sel, in_=sel_d)
    nc.sync.dma_start(out=idb, in_=idb_d)
    nc.sync.dma_start(out=idf, in_=idf_d)

    # per-half persistent buffers
    ssq_cols = singles.tile([128, 2 * n_supers], DT.float32, tag="ssq_cols", name="ssq_cols")
    stats_h = [singles.tile([128, 15 * W], DT.float32, tag=f"stats{h}", name=f"stats{h}") for h in range(2)]
    loss_h = [singles.tile([128, W], DT.float32, tag=f"loss{h}", name=f"loss{h}") for h in range(2)]

    for s in range(n_supers):
        h = s // half_supers
        sl = s % half_supers  # super index within half

        # ---- load + cast f32 -> bf16 (SWDGE) ----
        xb = loads.tile([128, 4, F], DT.bfloat16, tag="xb")
        yb = loads.tile([128, 4, F], DT.bfloat16, tag="yb")
        nc.gpsimd.dma_start(
            out=xb, in_=x_d[512 * s:512 * (s + 1), :].rearrange("(t p) f -> p t f", p=128))
        nc.gpsimd.dma_start(
            out=yb, in_=y_d[512 * s:512 * (s + 1), :].rearrange("(t p) f -> p t f", p=128))

        # ---- global sum of squares partials (coarse: one column per super) ----
        jx = junkp.tile([128, 4, F], DT.bfloat16, tag="jx")
        jy = junkp.tile([128, 4, F], DT.bfloat16, tag="jy")
        nc.scalar.activation(out=jx, in_=xb, func=ACT.Square,
                             accum_out=ssq_cols[:, s:s + 1])
        nc.vector.scalar_tensor_tensor(
            out=jy, in0=yb, scalar=1.0, in1=yb, op0=ALU.mult, op1=ALU.mult,
            accum_out=ssq_cols[:, n_supers + s:n_supers + s + 1])

        # ---- transposes: [128b, 128i] -> [128i, 128b] planes in PSUM ----
        # one PSUM bank per j: x-plane in cols 0:512, y-plane in cols 512:1024
        pT = [psum.tile([128, 1024], DT.bfloat16, tag=f"pT{j}", name=f"pT{j}") for j in range(3)]
        for t in range(4):
            for j in range(3):
                nc.tensor.transpose(
                    out=pT[j][:, 128 * t:128 * (t + 1)], in_=xb[:, t, j::3], identity=idb)
                nc.tensor.transpose(
                    out=pT[j][:, 512 + 128 * t:512 + 128 * (t + 1)], in_=yb[:, t, j::3],
                    identity=idb)

        # ---- evacuate PSUM -> SBUF (split ACT / DVE) ----
        xT = [planes.tile([128, 512], DT.bfloat16, tag=f"xT{j}", name=f"xT{j}") for j in range(3)]
        yT = [planes.tile([128, 512], DT.bfloat16, tag=f"yT{j}", name=f"yT{j}") for j in range(3)]
        for j in range(3):
            nc.scalar.copy(out=xT[j], in_=pT[j][:, 0:512])
            nc.vector.tensor_copy(out=yT[j], in_=pT[j][:, 512:1024])

        # ---- cross products (DVE, bf16 2x) ----
        pr = {}
        for j in range(3):
            for k in range(3):
                p_ = prods.tile([128, 512], DT.bfloat16, tag=f"pr{j}{k}", name=f"pr{j}{k}")
                nc.vector.tensor_mul(p_, xT[j], yT[k])
                pr[(j, k)] = p_

        # ---- reduction matmuls into stats PSUM (quantity q -> partition q) ----
        # q = 3j+k: G_jk; q = 9+j: sx_j; q = 12+k: sy_k
        pstat = psum.tile([15, 512], DT.float32, tag="stats")
        for q in range(14, -1, -1):
            if q >= 12:
                rhs = yT[q - 12]
            elif q >= 9:
                rhs = xT[q - 9]
            else:
                rhs = pr[(q // 3, q % 3)]
            nc.tensor.matmul(
                out=pstat[0:q + 1, :], lhsT=sel[:, 127 - q:128], rhs=rhs,
                start=(q == 14), stop=(q == 0), skip_group_check=True)

        # ---- evacuate stats, transpose to [batch-partition, quantity] ----
        st_raw = statsp.tile([15, 512], DT.float32, tag="straw")
        nc.scalar.copy(out=st_raw, in_=pstat)
        pchunk = psum.tile([128, 60], DT.float32, tag="stats")
        for tp in range(4):
            nc.tensor.transpose(
                out=pchunk[:, 15 * tp:15 * (tp + 1)],
                in_=st_raw[0:15, 128 * tp:128 * (tp + 1)], identity=idf[0:15, 0:15])
        dstv = stats_h[h][:].rearrange("p (q t) -> p t q", q=15)[:, 4 * sl:4 * (sl + 1), :]
        srcv = pchunk[:].rearrange("p (t q) -> p t q", t=4)
        nc.vector.tensor_copy(out=dstv, in_=srcv)

        if sl == half_supers - 1:
            _final_math(nc, fin, stats_h[h], loss_h[h], W)
            nc.sync.dma_start(out=loss_d[:, h * W:(h + 1) * W], in_=loss_h[h])
    nc.sync.dma_start(out=ssq_d, in_=ssq_cols)


def _final_math(nc, fin, stats, loss, W):
    f32 = DT.float32
    V = nc.vector
    S = nc.scalar

    def T_(tag, mult=1):
        return fin.tile([128, mult * W], f32, tag=tag, name=tag)

    stats_ap = stats[:]
    base = stats_ap.offset
    part = list(stats_ap.ap[0])

    def q_ap(q, n=1):
        """contiguous view of quantities [q, q+n) : [128, n*W]"""
        return stats[:, q * W:(q + n) * W]

    def q_view(q, dims):
        return bass.AP(stats_ap.tensor, base + q * W, [part] + [list(d) for d in dims])

    inv_n = -1.0 / 128.0

    # --- C = G - sx sy^T / N ---
    sp9 = T_("sp9", 9)
    sx_b = q_view(9, [[W, 3], [0, 3], [1, W]])   # (j, k, T)
    sy_b = q_view(12, [[0, 3], [W, 3], [1, W]])
    V.tensor_tensor(out=sp9[:].rearrange("p (j k t) -> p j k t", j=3, k=3),
                    in0=sx_b, in1=sy_b, op=ALU.mult)
    C = T_("C", 9)
    V.scalar_tensor_tensor(out=C, in0=sp9, scalar=inv_n, in1=q_ap(0, 9),
                           op0=ALU.mult, op1=ALU.add)
    Cap = C[:]

    def C_(j, k):
        return C[:, (3 * j + k) * W:(3 * j + k + 1) * W]

    # --- l2 = ssx + ssy - (|sx|^2 + |sy|^2)/N ---
    sq6 = T_("sq6", 6)
    V.tensor_tensor(out=sq6, in0=q_ap(9, 6), in1=q_ap(9, 6), op=ALU.mult)
    sqsum = T_("sqsum")
    V.tensor_reduce(out=sqsum, in_=_bv(sq6[:], [[1, W], [W, 6]]), axis=mybir.AxisListType.X,
                    op=ALU.add)
    l2 = T_("l2")
    V.tensor_scalar_mul(l2, sqsum, inv_n)

    # --- I1 = sum C^2 ---
    csq = T_("csq", 9)
    V.tensor_tensor(out=csq, in0=C, in1=C, op=ALU.mult)
    I1 = T_("I1")
    V.tensor_reduce(out=I1, in_=_bv(csq[:], [[1, W], [W, 9]]), axis=mybir.AxisListType.X,
                    op=ALU.add)

    # --- M = C^T C (9 entries incl. dup), trM2 = sum M^2 ---
    P27 = T_("P27", 27)
    ca = _bv(Cap, [[3 * W, 3], [W, 3], [0, 3], [1, W]])
    cb = _bv(Cap, [[3 * W, 3], [0, 3], [W, 3], [1, W]])
    V.tensor_tensor(out=P27[:].rearrange("p (j a b t) -> p j a b t", j=3, a=3, b=3),
                    in0=ca, in1=cb, op=ALU.mult)
    M9 = T_("M9", 9)
    V.tensor_reduce(out=M9, in_=_bv(P27[:], [[3 * W, 3], [W, 3], [1, W], [9 * W, 3]]),
                    axis=mybir.AxisListType.X, op=ALU.add)
    msq = T_("msq", 9)
    V.tensor_tensor(out=msq, in0=M9, in1=M9, op=ALU.mult)
    trM2 = T_("trM2")
    V.tensor_reduce(out=trM2, in_=_bv(msq[:], [[1, W], [W, 9]]), axis=mybir.AxisListType.X,
                    op=ALU.add)

    # --- I2 = (I1^2 - trM2)/2 ---
    I1sq = T_("I1sq")
    V.tensor_tensor(out=I1sq, in0=I1, in1=I1, op=ALU.mult)
    trM2h = T_("trM2h")
    V.tensor_scalar_mul(trM2h, trM2, 0.5)
    I2 = T_("I2")
    V.scalar_tensor_tensor(out=I2, in0=I1sq, scalar=0.5, in1=trM2h,
                           op0=ALU.mult, op1=ALU.subtract)

    # --- det(C) ---
    ta = T_("ta")
    tb = T_("tb")
    det = T_("det")
    V.tensor_tensor(out=ta, in0=C_(1, 1), in1=C_(2, 2), op=ALU.mult)
    V.tensor_tensor(out=tb, in0=C_(1, 2), in1=C_(2, 1), op=ALU.mult)
    cof = T_("cof")
    V.tensor_tensor(out=cof, in0=ta, in1=tb, op=ALU.subtract)
    V.tensor_tensor(out=det, in0=C_(0, 0), in1=cof, op=ALU.mult)
    V.tensor_tensor(out=ta, in0=C_(1, 0), in1=C_(2, 2), op=ALU.mult)
    V.tensor_tensor(out=tb, in0=C_(1, 2), in1=C_(2, 0), op=ALU.mult)
    V.tensor_tensor(out=cof, in0=ta, in1=tb, op=ALU.subtract)
    V.tensor_tensor(out=cof, in0=C_(0, 1), in1=cof, op=ALU.mult)
    V.tensor_tensor(out=det, in0=det, in1=cof, op=ALU.subtract)
    V.tensor_tensor(out=ta, in0=C_(1, 0), in1=C_(2, 1), op=ALU.mult)
    V.tensor_tensor(out=tb, in0=C_(1, 1), in1=C_(2, 0), op=ALU.mult)
    V.tensor_tensor(out=cof, in0=ta, in1=tb, op=ALU.subtract)
    V.tensor_tensor(out=cof, in0=C_(0, 2), in1=cof, op=ALU.mult)
    V.tensor_tensor(out=det, in0=det, in1=cof, op=ALU.add)
    e3 = T_("e3")
    S.activation(out=e3, in_=det, func=ACT.Abs)

    # --- normalize: u = 3/I1 ---
    I1c = T_("I1c")
    V.tensor_scalar_max(I1c, I1, 1e-20)
    u = T_("u")
    V.reciprocal(out=u, in_=I1c)
    V.tensor_scalar_mul(u, u, 3.0)
    usq = T_("usq")
    V.tensor_tensor(out=usq, in0=u, in1=u, op=ALU.mult)
    I2n = T_("I2n")
    V.tensor_tensor(out=I2n, in0=I2, in1=usq, op=ALU.mult)
    V.tensor_scalar_max(I2n, I2n, 0.0)
    su = T_("su")
    S.activation(out=su, in_=u, func=ACT.Sqrt)
    e3n = T_("e3n")
    V.tensor_tensor(out=e3n, in0=e3, in1=u, op=ALU.mult)
    V.tensor_tensor(out=e3n, in0=e3n, in1=su, op=ALU.mult)
    E8 = T_("E8")
    V.tensor_scalar_mul(E8, e3n, 8.0)
    c0 = T_("c0")
    V.tensor_scalar(out=c0, in0=I2n, scalar1=-4.0, scalar2=9.0, op0=ALU.mult, op1=ALU.add)

    # --- Newton init: n = sqrt(3 + 2*sqrt(I2n)) ---
    b3 = fin.tile([128, 1], f32, tag="b3", name="b3")
    V.memset(b3, 3.0)
    sqi = T_("sqi")
    S.activation(out=sqi, in_=I2n, func=ACT.Sqrt)
    n = T_("n")
    S.activation(out=n, in_=sqi, func=ACT.Sqrt, bias=b3[:, 0:1], scale=2.0)

    # --- Newton iterations on n^4 - 6n^2 - 8 e3n n + c0 ---
    t1 = T_("t1")
    t3 = T_("t3")
    s1 = T_("s1")
    f0 = T_("f0")
    fv = T_("fv")
    av = T_("av")
    fp = T_("fp")
    rp = T_("rp")
    dd = T_("dd")
    for it in range(4):
        V.tensor_tensor(out=t1, in0=n, in1=n, op=ALU.mult)
        V.scalar_tensor_tensor(out=t3, in0=t1, scalar=-6.0, in1=n,
                               op0=ALU.add, op1=ALU.mult)  # (n^2-6)*n
        V.scalar_tensor_tensor(out=s1, in0=E8, scalar=-1.0, in1=t3,
                               op0=ALU.mult, op1=ALU.add)  # t3 - E8
        V.tensor_tensor(out=f0, in0=s1, in1=n, op=ALU.mult)
        V.tensor_tensor(out=fv, in0=f0, in1=c0, op=ALU.add)
        V.scalar_tensor_tensor(out=av, in0=n, scalar=3.0, in1=t3,
                               op0=ALU.mult, op1=ALU.add)  # n^3 - 3n
        V.scalar_tensor_tensor(out=fp, in0=av, scalar=4.0, in1=E8,
                               op0=ALU.mult, op1=ALU.subtract)  # 4n^3-12n-8e
        V.tensor_scalar_max(fp, fp, 1e-5)
        V.reciprocal(out=rp, in_=fp)
        V.tensor_tensor(out=dd, in0=fv, in1=rp, op=ALU.mult)
        V.tensor_tensor(out=n, in0=n, in1=dd, op=ALU.subtract)
        if it == 0:
            V.tensor_scalar_min(n, n, 3.01)
            V.tensor_scalar_max(n, n, 1.70)

    # --- un-normalize: s = sqrt(I1/3), one Newton refinement for sqrt accuracy ---
    vv = T_("vv")
    V.tensor_scalar_mul(vv, I1, 1.0 / 3.0)
    V.tensor_scalar_max(vv, vv, 1e-30)
    s0 = T_("s0")
    S.activation(out=s0, in_=vv, func=ACT.Sqrt)
    rs = T_("rs")
    V.reciprocal(out=rs, in_=s0)
    V.tensor_tensor(out=rs, in0=vv, in1=rs, op=ALU.mult)   # vv/s0
    V.tensor_tensor(out=rs, in0=rs, in1=s0, op=ALU.add)
    V.tensor_scalar_mul(rs, rs, 0.5)                       # refined sqrt

    # --- loss = l2 - 2 * n * s ---
    V.tensor_tensor(out=n, in0=n, in1=rs, op=ALU.mult)
    V.scalar_tensor_tensor(out=loss[:], in0=n, scalar=-2.0, in1=l2,
                           op0=ALU.mult, op1=ALU.add)


# ---------------------------------------------------------------------------
# host glue
# ---------------------------------------------------------------------------


class Runner:
    """Cached jitted shard_map executor for repeated invocations (timing)."""

    def __init__(self, nc, n_cores=N_CORES):
        import jax
        from jax.experimental.shard_map import shard_map
        from jax.sharding import Mesh, PartitionSpec
        from concourse import bass2jax
        from concourse import mybir as _mybir

        bass2jax.install_neuronx_cc_hook()
        self.nc = nc
        self.n_cores = n_cores
        partition_name = nc.partition_id_tensor.name if nc.partition_id_tensor else None
        in_names, out_names, out_avals, zero_outs = [], [], [], []
        for alloc in nc.m.functions[0].allocations:
            if not isinstance(alloc, _mybir.MemoryLocationSet):
                continue
            name = alloc.memorylocations[0].name
            if alloc.kind == "ExternalInput":
                if name != partition_name:
                    in_names.append(name)
            elif alloc.kind == "ExternalOutput":
                out_names.append(name)
                shape = tuple(alloc.tensor_shape)
                dtype = _mybir.dt.np(alloc.dtype)
                out_avals.append(jax.core.ShapedArray(shape, dtype))
                zero_outs.append(np.zeros(shape, dtype))
        self.in_names = list(in_names)
        self.out_names = out_names
        self.zero_outs = zero_outs
        n_params = len(in_names)
        n_outs = len(out_avals)
        all_in_names = in_names + out_names
        if partition_name is not None:
            all_in_names = all_in_names + [partition_name]

        def _body(*args):
            operands = list(args)
            if partition_name is not None:
                operands.append(bass2jax.partition_id_tensor())
            outs = bass2jax._bass_exec_p.bind(
                *operands,
                out_avals=tuple(out_avals),
                in_names=tuple(all_in_names),
                out_names=tuple(out_names),
                lowering_input_output_aliases=(),
                sim_require_finite=True,
                sim_require_nnan=True,
                nc=nc,
            )
            return tuple(outs)

        devices = jax.devices()[:n_cores]
        mesh = Mesh(np.asarray(devices), ("core",))
        self.mesh = mesh
        in_specs = (PartitionSpec("core"),) * (n_params + n_outs)
        out_specs = (PartitionSpec("core"),) * n_outs
        self.fn = jax.jit(
            shard_map(_body, mesh=mesh, in_specs=in_specs, out_specs=out_specs,
                      check_rep=False),
            keep_unused=True,
        )

    def prep(self, in_maps, device_put=True):
        """in_maps: list of per-core dicts -> concatenated arg list (device-resident)."""
        concat = [
            np.concatenate([np.asarray(in_maps[c][n]) for c in range(self.n_cores)], axis=0)
            for n in self.in_names
        ]
        concat += [
            np.zeros((self.n_cores * z.shape[0], *z.shape[1:]), z.dtype)
            for z in self.zero_outs
        ]
        if device_put:
            import jax
            from jax.sharding import NamedSharding, PartitionSpec

            sh = NamedSharding(self.mesh, PartitionSpec("core"))
            concat = [jax.device_put(a, sh) for a in concat]
            jax.block_until_ready(concat)
        return concat

    def __call__(self, args):
        return self.fn(*args)


_NC_CACHE = {}


def _get_nc(b_core=B_CORE):
    if b_core not in _NC_CACHE:
        _NC_CACHE[b_core] = build_kernel(b_core)
    return _NC_CACHE[b_core]


def _consts():
    sel = np.zeros((128, 128), ml_dtypes.bfloat16)
    sel[:, 127] = 1.0
    idb = np.eye(128, dtype=ml_dtypes.bfloat16)
    idf = np.eye(128, dtype=np.float32)
    return sel, idb, idf


def run_cores(x, y, b_core=B_CORE, n_cores=N_CORES, nc=None):
    """x, y: (n_cores*b_core, 128, 3) float32 -> list of per-core loss grids."""
    if nc is None:
        nc = _get_nc(b_core)
    sel, idb, idf = _consts()
    xs = np.ascontiguousarray(x, dtype=np.float32).reshape(n_cores, b_core, F)
    ys = np.ascontiguousarray(y, dtype=np.float32).reshape(n_cores, b_core, F)
    in_maps = [
        {"x": xs[c], "y": ys[c], "sel": sel, "idb": idb, "idf": idf}
        for c in range(n_cores)
    ]
    res = run_bass_kernel_spmd(nc, in_maps, core_ids=list(range(n_cores)))
    return [(res.results[c]["loss"], res.results[c]["ssq"]) for c in range(n_cores)]


def kernel(x, y):
    """Full-input entry point: x, y (65536, 128, 3) float32 -> scalar float32."""
    grids = run_cores(np.asarray(x), np.asarray(y))
    total = sum(
        g.astype(np.float64).sum() + q.astype(np.float64).sum() for g, q in grids
    )
    return np.float32(total / (B_TOTAL * N_PTS * 3))

